# revision 1
# baseline (speedup 1.0000x reference)
"""CPC InfoNCE loss kernel for Trainium2 (8 NeuronCores, data-parallel rows).

Per core (rows sharded across cores, 3 horizons x 8 blocks of 128 rows):
  - Host normalizes the pool table all_z = normalize(z_seq.reshape(BT, D)) and
    uploads it transposed in bf16 (AZT). Host also gathers per-core anchor and
    positive rows (transposed, bf16), the predictor weights (transposed, bf16),
    and a dense per-row count matrix C [row, pool] (bf16; multiplicity of each
    pool entry among the row's 128 sampled negatives, plus 1 at the positive).
  - PE computes U^T = W @ Z_anchor^T, per-row norms ||u||^2 via a ones-matmul,
    and the positive logits via a ones-matmul over ut*az_pos products.
  - For each 128-row block PE computes the full similarity block
    S = U_blk @ AZT into PSUM; ACT applies exp(scale*S) straight out of PSUM
    (scale = 1/(tau*||u||) per row) into a bf16 SBUF tile.
  - DVE multiplies by the C tile (zeroing the ~98.4% unsampled entries,
    weighting duplicates) and reduces each row to R = sum_j e^{s_j}
    (positive included via its count). loss = ln(R) - s_pos per row.
  - Host averages the returned [128, 24] per-row losses with the horizon
    weights (the unshard step).
"""

import sys

sys.path.insert(0, "/opt/trn_rl_repo")

import math
import os

import ml_dtypes
import numpy as np

import concourse.bass as bass
import concourse.tile as tile
from concourse import bacc
from concourse import mybir
from concourse.bass_utils import run_bass_kernel_spmd

# Problem constants (hardcoded per contract)
B, T, D = 16, 512, 256
BT = B * T  # 8192 pool entries
HORIZONS = (1, 5, 21)
H = len(HORIZONS)
N_NEG = 128
TAU = 0.07
N_CORES = 8

P = 128
NROW = 1024  # padded rows per core per horizon
NBLK = NROW // P  # 8
NCOL = H * NBLK  # 24 row-blocks per core
POOL_TILE = 512
N_PTILES = BT // POOL_TILE  # 16

BF16 = mybir.dt.bfloat16
F32 = mybir.dt.float32


def _split_multiwait_drains(nc):
    """This walrus build accepts only one sync-wait command per TPB_CTRL
    instruction; TileContext's exit drain carries one wait per live proc.
    Split the extras into preceding single-wait drains."""
    for f in nc.m.functions:
        for bb in f.blocks:
            new_list = []
            for inst in bb.instructions:
                si = inst.sync_info
                if si is not None and si.on_wait and len(si.on_wait) > 1:
                    waits = list(si.on_wait)
                    for j, w in enumerate(waits[:-1]):
                        d = mybir.InstDrain(
                            name=f"{inst.name}-w{j}", ins=[], outs=[]
                        )
                        d.engine = inst.engine
                        d.sync_info = mybir.SyncInfo(on_wait=[w], on_update=[])
                        nc.register_instruction(d)
                        new_list.append(d)
                    si.on_wait = [waits[-1]]
                    inst.sync_info = si
                new_list.append(inst)
            bb.instructions[:] = new_list


def build_program(reps=1):
    reps = int(os.environ.get("KERNEL_REPS", reps))
    nc = bacc.Bacc(
        "TRN2", target_bir_lowering=False, debug=False, num_devices=N_CORES
    )

    azt_d = nc.declare_dram_parameter("azt", [P, 2, BT], BF16, isOutput=False)
    zat_d = nc.declare_dram_parameter("zat", [P, H * 2, NROW], BF16, isOutput=False)
    azp_d = nc.declare_dram_parameter("azp", [P, H * 2, NROW], BF16, isOutput=False)
    pt_d = nc.declare_dram_parameter("pt", [P, H * 4, P], BF16, isOutput=False)
    cnt_d = nc.declare_dram_parameter("cnt", [P, NCOL, BT], BF16, isOutput=False)
    loss_d = nc.declare_dram_parameter("loss", [P, NCOL], F32, isOutput=True)

    from contextlib import ExitStack, nullcontext

    with tile.TileContext(nc) as tc, ExitStack() as ctx:
        singles = ctx.enter_context(tc.tile_pool(name="singles", bufs=1))
        ut_pool = ctx.enter_context(tc.tile_pool(name="ut", bufs=2))
        c_pool = ctx.enter_context(tc.tile_pool(name="c", bufs=2))
        e_pool = ctx.enter_context(tc.tile_pool(name="e", bufs=2))
        small = ctx.enter_context(tc.tile_pool(name="small", bufs=2))
        junk_pool = ctx.enter_context(tc.tile_pool(name="junk", bufs=1))
        psum_s = ctx.enter_context(tc.tile_pool(name="psum_s", bufs=2, space="PSUM"))
        psum_u = ctx.enter_context(tc.tile_pool(name="psum_u", bufs=1, space="PSUM"))
        psum_b = ctx.enter_context(tc.tile_pool(name="psum_b", bufs=1, space="PSUM"))
        psum_r = ctx.enter_context(tc.tile_pool(name="psum_r", bufs=1, space="PSUM"))

        # ---- preload constants -------------------------------------------
        azt_sb = singles.tile([P, 2, BT], BF16)
        nc.sync.dma_start(out=azt_sb[:], in_=azt_d[:])
        zat_sb = singles.tile([P, H * 2, NROW], BF16)
        nc.sync.dma_start(out=zat_sb[:], in_=zat_d[:])
        azp_sb = singles.tile([P, H * 2, NROW], BF16)
        nc.sync.dma_start(out=azp_sb[:], in_=azp_d[:])
        pt_sb = singles.tile([P, H * 4, P], BF16)
        nc.sync.dma_start(out=pt_sb[:], in_=pt_d[:])

        ones_sb = singles.tile([P, 1], BF16)
        nc.vector.memset(ones_sb[:], 1.0)
        one1_sb = singles.tile([1, 1], F32)
        nc.vector.memset(one1_sb[:], 1.0)

        loss_sb = singles.tile([P, NCOL], F32)
        rsum_sb = singles.tile([P, NCOL], F32)
        rsT_sb = singles.tile([P, NCOL], F32)
        spT_sb = singles.tile([P, NCOL], F32)

        loop_cm = tc.For_i(0, reps, 1) if reps > 1 else nullcontext()
        with loop_cm:
            for i in range(H):
                # ---- predictions U^T + per-row norm / positive logit -----
                ut_sb = ut_pool.tile([P, 2, NROW], BF16, tag="ut")
                rs_flat = small.tile([1, NROW], F32, tag="rsflat")
                sp_flat = small.tile([1, NROW], F32, tag="spflat")
                nsum = small.tile([1, NROW], F32, tag="nsum")
                for mc in range(2):
                    for nh in range(2):  # one PSUM bank per matmul
                        nsl = slice(nh * (NROW // 2), (nh + 1) * (NROW // 2))
                        pu = psum_u.tile([P, NROW // 2], F32, tag="pu")
                        for kc in range(2):
                            nc.tensor.matmul(
                                pu[:],
                                pt_sb[:, i * 4 + kc * 2 + mc, :],
                                zat_sb[:, i * 2 + kc, nsl],
                                start=(kc == 0),
                                stop=(kc == 1),
                            )
                        # bf16 copy for the S-matmul lhsT
                        nc.scalar.copy(out=ut_sb[:, mc, nsl], in_=pu[:])
                    # squared entries (from the bf16-rounded values used below)
                    usq = junk_pool.tile([P, NROW], BF16, tag="usq")
                    nc.vector.tensor_mul(usq[:], ut_sb[:, mc, :], ut_sb[:, mc, :])
                    # ut * az_pos products for the positive logits
                    upr = junk_pool.tile([P, NROW], BF16, tag="upr")
                    nc.vector.tensor_mul(
                        upr[:], ut_sb[:, mc, :], azp_sb[:, i * 2 + mc, :]
                    )
                    # column sums via ones-matmuls, accumulated in SBUF
                    for nh in range(2):
                        nsl = slice(nh * (NROW // 2), (nh + 1) * (NROW // 2))
                        pb_n = psum_b.tile([1, NROW // 2], F32, tag="pbn")
                        pb_p = psum_b.tile([1, NROW // 2], F32, tag="pbp")
                        nc.tensor.matmul(
                            pb_n[:], ones_sb[:], usq[:, nsl],
                            start=True, stop=True,
                        )
                        nc.tensor.matmul(
                            pb_p[:], ones_sb[:], upr[:, nsl],
                            start=True, stop=True,
                        )
                        if mc == 0:
                            nc.vector.tensor_copy(out=nsum[0:1, nsl], in_=pb_n[:])
                            nc.vector.tensor_copy(out=sp_flat[0:1, nsl], in_=pb_p[:])
                        else:
                            nc.vector.tensor_add(
                                out=nsum[0:1, nsl], in0=nsum[0:1, nsl], in1=pb_n[:]
                            )
                            nc.vector.tensor_add(
                                out=sp_flat[0:1, nsl], in0=sp_flat[0:1, nsl],
                                in1=pb_p[:],
                            )
                # rs_flat = 1/(tau*||u||) = 1/sqrt(tau^2 * ||u||^2)
                nc.scalar.activation(
                    out=rs_flat[:], in_=nsum[:],
                    func=mybir.ActivationFunctionType.Sqrt,
                    scale=float(TAU * TAU),
                )
                nc.vector.reciprocal(out=rs_flat[:], in_=rs_flat[:])
                # sp_flat = raw_pos_dot * rs  (the positive logit)
                nc.vector.tensor_mul(sp_flat[:], sp_flat[:], rs_flat[:])
                # transpose the per-row scalars into per-block columns
                for rb in range(NBLK):
                    col = i * NBLK + rb
                    pr = psum_r.tile([P, 2], F32, tag="pr")
                    nc.tensor.matmul(
                        pr[:, 0:1], rs_flat[0:1, rb * P:(rb + 1) * P],
                        one1_sb[:], start=True, stop=True,
                    )
                    nc.tensor.matmul(
                        pr[:, 1:2], sp_flat[0:1, rb * P:(rb + 1) * P],
                        one1_sb[:], start=True, stop=True,
                    )
                    nc.scalar.copy(out=rsT_sb[:, col:col + 1], in_=pr[:, 0:1])
                    nc.scalar.copy(out=spT_sb[:, col:col + 1], in_=pr[:, 1:2])

                # ---- per row-block: S matmul -> exp -> masked reduce -----
                for rb in range(NBLK):
                    col = i * NBLK + rb
                    c_sb = c_pool.tile([P, BT], BF16, tag="c")
                    nc.sync.dma_start(out=c_sb[:], in_=cnt_d[:, col, :])
                    e_sb = e_pool.tile([P, BT], BF16, tag="e")
                    for ph in range(N_PTILES // 2):
                        ps = psum_s.tile([P, 2 * POOL_TILE], F32, tag="ps")
                        for sub in range(2):
                            pt_i = ph * 2 + sub
                            for kc in range(2):
                                nc.tensor.matmul(
                                    ps[:, sub * POOL_TILE:(sub + 1) * POOL_TILE],
                                    ut_sb[:, kc, rb * P:(rb + 1) * P],
                                    azt_sb[:, kc,
                                           pt_i * POOL_TILE:(pt_i + 1) * POOL_TILE],
                                    start=(kc == 0),
                                    stop=(kc == 1),
                                )
                        # exp straight out of PSUM (fused copy+scale+exp)
                        nc.scalar.activation(
                            out=e_sb[:, ph * 2 * POOL_TILE:(ph + 1) * 2 * POOL_TILE],
                            in_=ps[:],
                            func=mybir.ActivationFunctionType.Exp,
                            scale=rsT_sb[:, col:col + 1],
                        )
                    # R = sum_m cnt[m] * e[m]  (counts include the positive),
                    # fused multiply + free-dim accumulate on DVE
                    nc.vector.scalar_tensor_tensor(
                        out=e_sb[:], in0=e_sb[:], scalar=1.0, in1=c_sb[:],
                        op0=mybir.AluOpType.mult, op1=mybir.AluOpType.mult,
                        accum_out=rsum_sb[:, col:col + 1],
                    )
            # loss = ln(R) - s_pos, batched over all 24 columns
            nc.scalar.activation(
                out=loss_sb[:], in_=rsum_sb[:],
                func=mybir.ActivationFunctionType.Ln,
            )
            nc.vector.tensor_tensor(
                loss_sb[:], loss_sb[:], spT_sb[:], mybir.AluOpType.subtract,
            )

        nc.sync.dma_start(out=loss_d[:], in_=loss_sb[:])

    nc.compile()
    _split_multiwait_drains(nc)
    return nc


def prepare_inputs(z_seq, preds, neg_idx):
    """Host-side sharding/packing. Returns (in_maps, valid_counts)."""
    z_flat = np.asarray(z_seq, dtype=np.float32).reshape(BT, D)
    preds = np.asarray(preds, dtype=np.float32)
    neg_idx = np.asarray(neg_idx)

    norms = np.linalg.norm(z_flat, axis=1, keepdims=True)
    az = z_flat / np.maximum(norms, 1e-12)
    azt = np.ascontiguousarray(
        az.T.reshape(2, P, BT).transpose(1, 0, 2)
    ).astype(ml_dtypes.bfloat16)

    # pt[d, i*4+kc*2+mc, e] = preds[i, mc*128+e, kc*128+d]
    pt = np.empty((P, H * 4, P), dtype=ml_dtypes.bfloat16)
    for i in range(H):
        w = preds[i]  # [e_out, d_in]
        for kc in range(2):
            for mc in range(2):
                blk = w[mc * P:(mc + 1) * P, kc * P:(kc + 1) * P]  # [e, d]
                pt[:, i * 4 + kc * 2 + mc, :] = blk.T.astype(ml_dtypes.bfloat16)

    in_maps = []
    valid_counts = np.zeros((N_CORES, H), dtype=np.int64)
    for c in range(N_CORES):
        n0 = c * NROW
        zat = np.zeros((P, H * 2, NROW), dtype=ml_dtypes.bfloat16)
        azp = np.zeros((P, H * 2, NROW), dtype=ml_dtypes.bfloat16)
        cnt = np.zeros((P, NCOL, BT), dtype=ml_dtypes.bfloat16)
        for i, k in enumerate(HORIZONS):
            L = T - k
            BL = B * L
            nvalid = min(max(BL - n0, 0), NROW)
            valid_counts[c, i] = nvalid
            n = n0 + np.arange(NROW)
            nv = n[:nvalid]
            b = nv // L
            a_full = np.zeros(NROW, dtype=np.int64)
            a_full[:nvalid] = nv + b * k          # anchor flat rows
            p_full = np.zeros(NROW, dtype=np.int64)
            p_full[:nvalid] = nv + (b + 1) * k    # positive flat rows
            zat[:, i * 2:(i + 1) * 2, :] = (
                z_flat[a_full].T.reshape(2, P, NROW).transpose(1, 0, 2)
            ).astype(ml_dtypes.bfloat16)
            azp[:, i * 2:(i + 1) * 2, :] = (
                az[p_full].T.reshape(2, P, NROW).transpose(1, 0, 2)
            ).astype(ml_dtypes.bfloat16)

            # dense counts: negatives multiplicity + 1 at the positive
            cm = np.zeros((NROW, BT), dtype=np.float32)
            rows = np.repeat(np.arange(nvalid), N_NEG)
            np.add.at(cm, (rows, neg_idx[i, nv, :].reshape(-1)), 1.0)
            cm[np.arange(NROW), p_full] += 1.0
            if nvalid < NROW:
                # pad rows: keep a single count so R>0 (host ignores them)
                cm[nvalid:] = 0.0
                cm[nvalid:, 0] = 1.0
            cmb = cm.astype(ml_dtypes.bfloat16)
            for rb in range(NBLK):
                cnt[:, i * NBLK + rb, :] = cmb[rb * P:(rb + 1) * P]

        in_maps.append({"azt": azt, "zat": zat, "azp": azp, "pt": pt, "cnt": cnt})
    return in_maps, valid_counts


def reduce_outputs(results, valid_counts):
    raw_w = {k: 1.0 / math.sqrt(k) for k in HORIZONS}
    tot_w = sum(raw_w.values())
    total = np.float64(0.0)
    for i, k in enumerate(HORIZONS):
        L = T - k
        BL = B * L
        s = np.float64(0.0)
        for c in range(N_CORES):
            nvalid = int(valid_counts[c, i])
            if nvalid == 0:
                continue
            lm = results[c]["loss"]  # [P, NCOL]
            per_row = lm[:, i * NBLK:(i + 1) * NBLK].T.reshape(NROW)
            s += per_row[:nvalid].sum(dtype=np.float64)
        total += (raw_w[k] / tot_w) * (s / BL)
    return np.float32(total)


_CACHED_NC = None


def kernel(z_seq, preds, neg_idx):
    global _CACHED_NC
    if _CACHED_NC is None:
        _CACHED_NC = build_program()
    nc = _CACHED_NC
    in_maps, valid_counts = prepare_inputs(z_seq, preds, neg_idx)
    res = run_bass_kernel_spmd(nc, in_maps, list(range(N_CORES)))
    return reduce_outputs(res.results, valid_counts)


if __name__ == "__main__":
    rng = np.random.default_rng(0)
    z = rng.standard_normal((B, T, D), dtype=np.float32)
    pr = (rng.standard_normal((H, D, D), dtype=np.float32) / np.sqrt(D)).astype(
        np.float32
    )
    ni = rng.integers(0, BT, size=(H, BT, N_NEG), dtype=np.int64)
    print(kernel(z, pr, ni))



# revision 2
# speedup vs baseline: 1.4052x; 1.4052x over previous
"""CPC InfoNCE loss kernel for Trainium2 — baseline structure + fp8 DoubleRow
S-matmuls.

Identical pipeline to the proven baseline (dense S per 128-row block, ACT exp
drain, DVE count-mask reduce, host-packed dense counts), with the similarity
matmul switched to fp8e4m3 DoubleRow (K=256 folded into one instruction,
2x PE throughput) and the pool table uploaded in fp8 (half the preload DMA).
u is kept in bf16 for the norm/positive-logit path (exactness) and quantized
to fp8 only as the S-matmul lhsT.
"""

import sys

sys.path.insert(0, "/opt/trn_rl_repo")

import math
import os

import ml_dtypes
import numpy as np

import concourse.bass as bass
import concourse.tile as tile
from concourse import bacc
from concourse import mybir
from concourse.bass_utils import run_bass_kernel_spmd

# Problem constants (hardcoded per contract)
B, T, D = 16, 512, 256
BT = B * T  # 8192 pool entries
HORIZONS = (1, 5, 21)
H = len(HORIZONS)
N_NEG = 128
TAU = 0.07
N_CORES = 8

P = 128
NROW = 1024  # padded rows per core per horizon
NBLK = NROW // P  # 8
NCOL = H * NBLK  # 24 row-blocks per core
POOL_TILE = 512
N_PTILES = BT // POOL_TILE  # 16

BF16 = mybir.dt.bfloat16
F32 = mybir.dt.float32
FP8 = mybir.dt.float8e4


def _split_multiwait_drains(nc):
    """This walrus build accepts only one sync-wait command per TPB_CTRL
    instruction; TileContext's exit drain carries one wait per live proc.
    Split the extras into preceding single-wait drains."""
    for f in nc.m.functions:
        for bb in f.blocks:
            new_list = []
            for inst in bb.instructions:
                si = inst.sync_info
                if si is not None and si.on_wait and len(si.on_wait) > 1:
                    waits = list(si.on_wait)
                    for j, w in enumerate(waits[:-1]):
                        d = mybir.InstDrain(
                            name=f"{inst.name}-w{j}", ins=[], outs=[]
                        )
                        d.engine = inst.engine
                        d.sync_info = mybir.SyncInfo(on_wait=[w], on_update=[])
                        nc.register_instruction(d)
                        new_list.append(d)
                    si.on_wait = [waits[-1]]
                    inst.sync_info = si
                new_list.append(inst)
            bb.instructions[:] = new_list


def build_program(reps=1):
    reps = int(os.environ.get("KERNEL_REPS", reps))
    nc = bacc.Bacc(
        "TRN2", target_bir_lowering=False, debug=False, num_devices=N_CORES
    )

    azt_d = nc.declare_dram_parameter("azt", [P, 2, BT], FP8, isOutput=False)
    zat_d = nc.declare_dram_parameter("zat", [P, H * 2, NROW], BF16, isOutput=False)
    azp_d = nc.declare_dram_parameter("azp", [P, H * 2, NROW], BF16, isOutput=False)
    pt_d = nc.declare_dram_parameter("pt", [P, H * 4, P], BF16, isOutput=False)
    cnt_d = nc.declare_dram_parameter("cnt", [P, NCOL, BT], BF16, isOutput=False)
    loss_d = nc.declare_dram_parameter("loss", [P, NCOL], F32, isOutput=True)

    from contextlib import ExitStack, nullcontext

    with tile.TileContext(nc) as tc, ExitStack() as ctx:
        singles = ctx.enter_context(tc.tile_pool(name="singles", bufs=1))
        ut_pool = ctx.enter_context(tc.tile_pool(name="ut", bufs=2))
        c_pool = ctx.enter_context(tc.tile_pool(name="c", bufs=2))
        e_pool = ctx.enter_context(tc.tile_pool(name="e", bufs=2))
        small = ctx.enter_context(tc.tile_pool(name="small", bufs=2))
        junk_pool = ctx.enter_context(tc.tile_pool(name="junk", bufs=1))
        psum_s = ctx.enter_context(tc.tile_pool(name="psum_s", bufs=4, space="PSUM"))

        # ---- preload constants (scalar HWDGE ring; cnt streams on sync) --
        azt_sb = singles.tile([P, 2, BT], FP8)
        nc.sync.dma_start(out=azt_sb[:], in_=azt_d[:])
        zat_sb = singles.tile([P, H * 2, NROW], BF16)
        nc.sync.dma_start(out=zat_sb[:], in_=zat_d[:])
        azp_sb = singles.tile([P, H * 2, NROW], BF16)
        nc.sync.dma_start(out=azp_sb[:], in_=azp_d[:])
        pt_sb = singles.tile([P, H * 4, P], BF16)
        nc.sync.dma_start(out=pt_sb[:], in_=pt_d[:])

        ones_sb = singles.tile([P, 1], BF16)
        nc.vector.memset(ones_sb[:], 1.0)
        one1_sb = singles.tile([1, 1], F32)
        nc.vector.memset(one1_sb[:], 1.0)

        loss_sb = singles.tile([P, NCOL], F32)
        rsum_sb = singles.tile([P, NCOL], F32)
        rsT_sb = singles.tile([P, NCOL], F32)
        spT_sb = singles.tile([P, NCOL], F32)

        ut8s = [None] * H

        def emit_u_phase(i):
            # ---- predictions U^T + per-row norm / positive logit -----
            ut16 = ut_pool.tile([P, 2, NROW], BF16, tag="ut16")
            ut8 = ut_pool.tile([P, 2, NROW], FP8, tag="ut8")
            ut8s[i] = ut8
            rs_flat = small.tile([1, NROW], F32, tag="rsflat")
            sp_flat = small.tile([1, NROW], F32, tag="spflat")
            nsum = small.tile([1, NROW], F32, tag="nsum")
            for mc in range(2):
                for nh in range(2):
                    nsl = slice(nh * (NROW // 2), (nh + 1) * (NROW // 2))
                    pu = psum_s.tile([P, 1024], F32, tag="ps")
                    for kc in range(2):
                        nc.tensor.matmul(
                            pu[:, 0:NROW // 2],
                            pt_sb[:, i * 4 + kc * 2 + mc, :],
                            zat_sb[:, i * 2 + kc, nsl],
                            start=(kc == 0),
                            stop=(kc == 1),
                        )
                    # bf16 copy (DVE) for norms, fp8 copy (ACT) for S-lhsT
                    nc.vector.tensor_copy(
                        out=ut16[:, mc, nsl], in_=pu[:, 0:NROW // 2]
                    )
                    nc.vector.tensor_copy(out=ut8[:, mc, nsl], in_=pu[:, 0:NROW // 2])
                # squared entries (bf16 values; norm of the pre-fp8 u)
                usq = junk_pool.tile([P, NROW], BF16, tag="usq")
                nc.vector.tensor_mul(usq[:], ut16[:, mc, :], ut16[:, mc, :])
                # ut * az_pos products for the positive logits
                upr = junk_pool.tile([P, NROW], BF16, tag="upr")
                nc.vector.tensor_mul(
                    upr[:], ut16[:, mc, :], azp_sb[:, i * 2 + mc, :]
                )
                # column sums via ones-matmuls, accumulated in SBUF
                for nh in range(2):
                    nsl = slice(nh * (NROW // 2), (nh + 1) * (NROW // 2))
                    pb = psum_s.tile([P, 1024], F32, tag="ps")
                    pb_n = pb[0:1, 0:NROW // 2]
                    pb_p = pb[0:1, NROW // 2:NROW]
                    nc.tensor.matmul(
                        pb_n, ones_sb[:], usq[:, nsl],
                        start=True, stop=True,
                    )
                    nc.tensor.matmul(
                        pb_p, ones_sb[:], upr[:, nsl],
                        start=True, stop=True,
                    )
                    if mc == 0:
                        nc.vector.tensor_copy(out=nsum[0:1, nsl], in_=pb_n)
                        nc.vector.tensor_copy(out=sp_flat[0:1, nsl], in_=pb_p)
                    else:
                        nc.vector.tensor_add(
                            out=nsum[0:1, nsl], in0=nsum[0:1, nsl], in1=pb_n
                        )
                        nc.vector.tensor_add(
                            out=sp_flat[0:1, nsl], in0=sp_flat[0:1, nsl],
                            in1=pb_p,
                        )
            # rs_flat = 1/(tau*||u||) = exp(-0.5*ln(tau^2*||u||^2));
            # ln+exp share one ACT table with the S-phase exps (no reloads,
            # unlike sqrt which always forces a table switch)
            lntmp = small.tile([1, NROW], F32, tag="lntmp")
            nc.scalar.activation(
                out=lntmp[:], in_=nsum[:],
                func=mybir.ActivationFunctionType.Ln,
                scale=float(TAU * TAU),
            )
            nc.scalar.activation(
                out=rs_flat[:], in_=lntmp[:],
                func=mybir.ActivationFunctionType.Exp,
                scale=-0.5,
            )
            # sp_flat = raw_pos_dot * rs  (the positive logit)
            nc.vector.tensor_mul(sp_flat[:], sp_flat[:], rs_flat[:])
            # transpose the per-row scalars into per-block columns: 16 tiny
            # matmuls into one PSUM slot, then two strided batched copies
            pr = psum_s.tile([P, 1024], F32, tag="ps")
            for rb in range(NBLK):
                nc.tensor.matmul(
                    pr[:, 2 * rb:2 * rb + 1], rs_flat[0:1, rb * P:(rb + 1) * P],
                    one1_sb[:], start=True, stop=True,
                )
                nc.tensor.matmul(
                    pr[:, 2 * rb + 1:2 * rb + 2],
                    sp_flat[0:1, rb * P:(rb + 1) * P],
                    one1_sb[:], start=True, stop=True,
                )
            csl = slice(i * NBLK, (i + 1) * NBLK)
            nc.vector.tensor_copy(out=rsT_sb[:, csl], in_=pr[:, 0:2 * NBLK:2])
            nc.vector.tensor_copy(out=spT_sb[:, csl], in_=pr[:, 1:2 * NBLK:2])

        def emit_s_block(i, rb):
            # ---- one row-block: S matmul -> exp -> masked reduce -----
            col = i * NBLK + rb
            ut8 = ut8s[i]
            c_sb = c_pool.tile([P, BT], BF16, tag="c")
            nc.sync.dma_start(out=c_sb[:], in_=cnt_d[:, col, :])
            e_sb = e_pool.tile([P, BT], BF16, tag="e")
            for ph in range(N_PTILES // 2):
                ps = psum_s.tile([P, 1024], F32, tag="ps")
                for sub in range(2):
                    pt_i = ph * 2 + sub
                    nc.tensor.matmul(
                        ps[:, sub * POOL_TILE:(sub + 1) * POOL_TILE],
                        ut8[:, :, rb * P:(rb + 1) * P],
                        azt_sb[:, :,
                               pt_i * POOL_TILE:(pt_i + 1) * POOL_TILE],
                        start=True, stop=True,
                        perf_mode=mybir.MatmulPerfMode.DoubleRow,
                    )
                # exp straight out of PSUM (fused copy+scale+exp)
                nc.scalar.activation(
                    out=e_sb[:, ph * 2 * POOL_TILE:(ph + 1) * 2 * POOL_TILE],
                    in_=ps[:],
                    func=mybir.ActivationFunctionType.Exp,
                    scale=rsT_sb[:, col:col + 1],
                )
            # R = sum_m cnt[m] * e[m]  (counts include the positive),
            # fused multiply + free-dim accumulate on DVE
            nc.vector.scalar_tensor_tensor(
                out=e_sb[:], in0=e_sb[:], scalar=1.0, in1=c_sb[:],
                op0=mybir.AluOpType.mult, op1=mybir.AluOpType.mult,
                accum_out=rsum_sb[:, col:col + 1],
            )

        loop_cm = tc.For_i(0, reps, 1) if reps > 1 else nullcontext()
        with loop_cm:
            # software-pipelined: horizon i+1's U phase is emitted between
            # the first and second S blocks of horizon i
            emit_u_phase(0)
            for i in range(H):
                for rb in range(NBLK):
                    emit_s_block(i, rb)
                    if rb == 0 and i + 1 < H:
                        emit_u_phase(i + 1)
            # loss = ln(R) - s_pos, batched over all 24 columns
            nc.scalar.activation(
                out=loss_sb[:], in_=rsum_sb[:],
                func=mybir.ActivationFunctionType.Ln,
            )
            nc.vector.tensor_tensor(
                loss_sb[:], loss_sb[:], spT_sb[:], mybir.AluOpType.subtract,
            )

        nc.sync.dma_start(out=loss_d[:], in_=loss_sb[:])

    nc.compile()
    _split_multiwait_drains(nc)
    return nc


def prepare_inputs(z_seq, preds, neg_idx):
    """Host-side sharding/packing. Returns (in_maps, valid_counts)."""
    z_flat = np.asarray(z_seq, dtype=np.float32).reshape(BT, D)
    preds = np.asarray(preds, dtype=np.float32)
    neg_idx = np.asarray(neg_idx)

    norms = np.linalg.norm(z_flat, axis=1, keepdims=True)
    az = z_flat / np.maximum(norms, 1e-12)
    azt = np.ascontiguousarray(
        az.T.reshape(2, P, BT).transpose(1, 0, 2)
    )
    azt8 = np.clip(azt, -240, 240).astype(ml_dtypes.float8_e4m3)

    # pt[d, i*4+kc*2+mc, e] = preds[i, mc*128+e, kc*128+d]
    pt = np.empty((P, H * 4, P), dtype=ml_dtypes.bfloat16)
    for i in range(H):
        w = preds[i]  # [e_out, d_in]
        for kc in range(2):
            for mc in range(2):
                blk = w[mc * P:(mc + 1) * P, kc * P:(kc + 1) * P]  # [e, d]
                pt[:, i * 4 + kc * 2 + mc, :] = blk.T.astype(ml_dtypes.bfloat16)

    in_maps = []
    valid_counts = np.zeros((N_CORES, H), dtype=np.int64)
    for c in range(N_CORES):
        n0 = c * NROW
        zat = np.zeros((P, H * 2, NROW), dtype=ml_dtypes.bfloat16)
        azp = np.zeros((P, H * 2, NROW), dtype=ml_dtypes.bfloat16)
        cnt = np.zeros((P, NCOL, BT), dtype=ml_dtypes.bfloat16)
        for i, k in enumerate(HORIZONS):
            L = T - k
            BL = B * L
            nvalid = min(max(BL - n0, 0), NROW)
            valid_counts[c, i] = nvalid
            n = n0 + np.arange(NROW)
            nv = n[:nvalid]
            b = nv // L
            a_full = np.zeros(NROW, dtype=np.int64)
            a_full[:nvalid] = nv + b * k          # anchor flat rows
            p_full = np.zeros(NROW, dtype=np.int64)
            p_full[:nvalid] = nv + (b + 1) * k    # positive flat rows
            zat[:, i * 2:(i + 1) * 2, :] = (
                z_flat[a_full].T.reshape(2, P, NROW).transpose(1, 0, 2)
            ).astype(ml_dtypes.bfloat16)
            azp[:, i * 2:(i + 1) * 2, :] = (
                az[p_full].T.reshape(2, P, NROW).transpose(1, 0, 2)
            ).astype(ml_dtypes.bfloat16)

            # dense counts: negatives multiplicity + 1 at the positive
            cm = np.zeros((NROW, BT), dtype=np.float32)
            rows = np.repeat(np.arange(nvalid), N_NEG)
            np.add.at(cm, (rows, neg_idx[i, nv, :].reshape(-1)), 1.0)
            cm[np.arange(NROW), p_full] += 1.0
            if nvalid < NROW:
                # pad rows: keep a single count so R>0 (host ignores them)
                cm[nvalid:] = 0.0
                cm[nvalid:, 0] = 1.0
            cmb = cm.astype(ml_dtypes.bfloat16)
            for rb in range(NBLK):
                cnt[:, i * NBLK + rb, :] = cmb[rb * P:(rb + 1) * P]

        in_maps.append({"azt": azt8, "zat": zat, "azp": azp, "pt": pt, "cnt": cnt})
    return in_maps, valid_counts


def reduce_outputs(results, valid_counts):
    raw_w = {k: 1.0 / math.sqrt(k) for k in HORIZONS}
    tot_w = sum(raw_w.values())
    total = np.float64(0.0)
    for i, k in enumerate(HORIZONS):
        L = T - k
        BL = B * L
        s = np.float64(0.0)
        for c in range(N_CORES):
            nvalid = int(valid_counts[c, i])
            if nvalid == 0:
                continue
            lm = results[c]["loss"]  # [P, NCOL]
            per_row = lm[:, i * NBLK:(i + 1) * NBLK].T.reshape(NROW)
            s += per_row[:nvalid].sum(dtype=np.float64)
        total += (raw_w[k] / tot_w) * (s / BL)
    return np.float32(total)


_CACHED_NC = None


def kernel(z_seq, preds, neg_idx):
    global _CACHED_NC
    if _CACHED_NC is None:
        _CACHED_NC = build_program()
    nc = _CACHED_NC
    in_maps, valid_counts = prepare_inputs(z_seq, preds, neg_idx)
    res = run_bass_kernel_spmd(nc, in_maps, list(range(N_CORES)))
    return reduce_outputs(res.results, valid_counts)


if __name__ == "__main__":
    rng = np.random.default_rng(0)
    z = rng.standard_normal((B, T, D), dtype=np.float32)
    pr = (rng.standard_normal((H, D, D), dtype=np.float32) / np.sqrt(D)).astype(
        np.float32
    )
    ni = rng.integers(0, BT, size=(H, BT, N_NEG), dtype=np.int64)
    print(kernel(z, pr, ni))
